# revision 40
# baseline (speedup 1.0000x reference)
"""Trainium2 Bass kernel for nn_ModAttn (modulated multi-function attention).

Shapes: x [1,1024,512], compatibility [1,4,1024]; out [1,4,1024,512].
Sharding: 8 cores = (function f in 0..3) x (head-half hh in 0..1). Each core
computes 4 heads (global heads hh*4..hh*4+3) over all 1024 queries/keys and
produces a PARTIAL output projection over its 256 d-dims; the host sums the
two partials per function (each core adds b_proj/2).

Per-core dataflow, attention in NORMAL orientation (query n on partitions):
  cm_q/cm_p   = layernorm(w_c @ code_f) modulation rows (phase A)
  C           = comp^T comp tiles [n-chunk 128, m 1024] bf16 (phase C)
  xm          = xT * cm_q (per-partition, 4x DVE)            (phase B)
  qT,kT       = [128=2 heads x 64d, 1024 tok] bf16; v dense [m-chunk, 256]
  per head (pipelined):
    S         = qT-slice^T @ kT-slice -> PSUM [128 n, 1024 m]
    e1        = ACT Exp(SCALE*S), accum_out -> s (softmax-1 denominators)
    t         = e1 * C[n-chunk]         (DVE 2x)
    e2        = ACT Exp(t * (1/s) per-partition scale), accum_out -> z2
    e2n       = e2 * (1/z2)             (DVE tensor_scalar 4x)
    e2T       = DMA-engine xbar transpose -> [m-local, m-chunk, n]
    ypv       = v^T @ e2T (PSUM accum over m-chunks)
    ymT       = ypv * cm_p-col          (DVE per-partition)
  y_partial   = ymT^T @ W_proj-slice + b_proj/2 -> HBM fp32   (phase E)
"""

import numpy as np
from contextlib import ExitStack

N_CORES = 8
N, DIN, NF, H, HD = 1024, 512, 4, 8, 64
HH = 4          # heads per core
DSL = 256       # qkv d-dims per core
SCALE = HD ** -0.5

_CACHE = {}


def build_nc():
    import concourse.bacc as bacc
    import concourse.tile as tile
    from concourse import mybir

    F32 = mybir.dt.float32
    BF16 = mybir.dt.bfloat16
    AT = mybir.ActivationFunctionType
    OP = mybir.AluOpType

    nc = bacc.Bacc("TRN2", target_bir_lowering=False, debug=False,
                   num_devices=N_CORES)

    xmT_d = nc.dram_tensor("xmT", [DIN, N], BF16, kind="ExternalInput")
    ct_d = nc.dram_tensor("ct", [N, N], BF16, kind="ExternalInput")
    wqkvT_d = nc.dram_tensor("wqkvT", [DIN, 3 * DSL], BF16,
                             kind="ExternalInput")
    wprojT_d = nc.dram_tensor("wprojT", [DSL, DIN], BF16,
                              kind="ExternalInput")
    aux_d = nc.dram_tensor("aux", [128, 12], F32, kind="ExternalInput")
    bv_d = nc.dram_tensor("bv", [1, DSL], BF16, kind="ExternalInput")
    bp2_d = nc.dram_tensor("bp2", [1, DIN], BF16, kind="ExternalInput")
    y_d = nc.dram_tensor("y", [N, DIN], F32, kind="ExternalOutput")

    with tile.TileContext(nc) as tc, ExitStack() as top:
        const = top.enter_context(tc.tile_pool(name="const", bufs=1))
        ones_r = const.tile([1, 128], F32, tag="ones_r")
        nc.vector.memset(ones_r[:], 1.0)
        ones_rb = const.tile([1, 128], BF16, tag="ones_rb")
        nc.vector.memset(ones_rb[:], 1.0)
        warm = const.tile([1, 1], F32, tag="warm")
        nc.scalar.activation(warm[:], ones_r[0:1, 0:1], AT.Exp)
        F32R = mybir.dt.float32r
        ones_rr = const.tile([1, 64], F32R, tag="ones_rr")
        nc.vector.memset(ones_rr[:].bitcast(F32), 1.0)

        big = top.enter_context(tc.tile_pool(name="big", bufs=1))
        xm = [big.tile([128, N], BF16, tag=f"xm{c}", name=f"xm{c}")
              for c in range(4)]
        wq = [big.tile([128, 3 * DSL], BF16, tag=f"wq{c}", name=f"wq{c}")
              for c in range(4)]
        wp = [big.tile([128, DIN], BF16, tag=f"wp{c}", name=f"wp{c}")
              for c in range(2)]
        for c in range(4):
            nc.gpsimd.dma_start(xm[c][:], xmT_d.ap()[c * 128:(c + 1) * 128, :])
        for c in range(4):
            nc.gpsimd.dma_start(wq[c][:], wqkvT_d.ap()[c * 128:(c + 1) * 128, :])
        aux_t = big.tile([128, 12], F32, tag="aux_t")
        nc.sync.dma_start(aux_t[:], aux_d.ap())

        bv_t = big.tile([1, DSL], BF16, tag="bv_t")
        nc.sync.dma_start(bv_t[:], bv_d.ap())
        bp2_t = big.tile([1, DIN], BF16, tag="bp2_t")
        nc.sync.dma_start(bp2_t[:], bp2_d.ap())
        bp2b = big.tile([128, DIN], BF16, tag="bp2b")
        nc.gpsimd.partition_broadcast(bp2b[:], bp2_t[:], channels=128)
        for c in range(2):
            nc.gpsimd.dma_start(wp[c][:], wprojT_d.ap()[c * 128:(c + 1) * 128, :])

        # persistent attention operands
        qkv = top.enter_context(tc.tile_pool(name="qkv", bufs=1))
        qT = [qkv.tile([128, N], BF16, tag=f"qT{j}", name=f"qT{j}")
              for j in range(2)]
        kT = [qkv.tile([128, N], BF16, tag=f"kT{j}", name=f"kT{j}")
              for j in range(2)]
        vv = [qkv.tile([128, HH * (HD + 1)], BF16, tag=f"vv{m}",
                       name=f"vv{m}") for m in range(8)]
        Ct = [qkv.tile([128, N], BF16, tag=f"Ct{n}", name=f"Ct{n}")
              for n in range(8)]
        ymT = [qkv.tile([128, N], BF16, tag=f"ymT{c}", name=f"ymT{c}")
               for c in range(2)]
        e2T = [qkv.tile([128, 8, N], BF16, tag=f"e2T{b}", name=f"e2T{b}")
               for b in range(3)]
        for ncb in range(8):
            nc.gpsimd.dma_start(Ct[ncb][:],
                                ct_d.ap()[ncb * 128:(ncb + 1) * 128, :])

        # ---------- phase B + D interleaved ----------
        mid = ExitStack()
        psS = mid.enter_context(tc.tile_pool(name="psS", bufs=3, space="PSUM"))
        psV = mid.enter_context(tc.tile_pool(name="psV", bufs=1, space="PSUM"))

        # warm the PE pstate with throwaway matmuls on resident const data
        wps = psS.tile([128, N], F32, tag="ps_s", name="ps_s")
        for r in range(6):
            nc.tensor.matmul(wps[:, 0:512], ones_rb[:], bp2b[0:1, :],
                             start=True, stop=True)

        qk0_ps = {}

        def emit_qk_half(j, which, half):
            dst, off, col = ((qT[j], j * 128, j) if which == "q"
                             else (kT[j], DSL + j * 128, 2 + j))
            key = (j, which)
            if half == 0:
                qk0_ps[key] = psS.tile([128, N], F32, tag="ps_s",
                                       name="ps_s")
            ps = qk0_ps[key]
            hs = slice(half * 512, (half + 1) * 512)
            for c in range(4):
                nc.tensor.matmul(ps[:, hs], wq[c][:, off:off + 128],
                                 xm[c][:, hs], start=(c == 0), stop=(c == 3))
            nc.vector.tensor_scalar_add(dst[:, hs], ps[:, hs],
                                        aux_t[:, col:col + 1])

        def emit_qk(j, engine):
            for half in range(2):
                emit_qk_half(j, "k", half)
                emit_qk_half(j, "q", half)

        # --- background work generators (emitted between D1 chunks) ---

        def bg_v(mc):
            def emit():
                ps = psS.tile([128, N], F32, tag="ps_s", name="ps_s")
                pv = ps[:, 0:DSL]
                for c in range(4):
                    nc.tensor.matmul(pv, xm[c][:, mc * 128:(mc + 1) * 128],
                                     wq[c][:, 2 * DSL:3 * DSL],
                                     start=(c == 0), stop=False)
                nc.tensor.matmul(pv, ones_rb[:], bv_t[:], start=False,
                                 stop=True)
                v3 = vv[mc][:].rearrange("p (h e) -> p h e", e=HD + 1)
                nc.vector.tensor_copy(
                    v3[:, :, 0:HD], pv.rearrange("p (h e) -> p h e", e=HD))
                nc.vector.memset(v3[:, :, HD:HD + 1], 1.0)
            return emit

        qk1_state = {}

        def bg_qk1(which, half):
            off = 128 if which == "q" else DSL + 128
            def emit():
                if half == 0:
                    qk1_state[which] = psS.tile([128, N], F32, tag="ps_s",
                                                name="ps_s")
                ps = qk1_state[which]
                for c in range(4):
                    nc.tensor.matmul(ps[:, half * 512:(half + 1) * 512],
                                     wq[c][:, off:off + 128],
                                     xm[c][:, half * 512:(half + 1) * 512],
                                     start=(c == 0), stop=(c == 3))
                if half == 1:
                    dst, col = (qT[1], 1) if which == "q" else (kT[1], 3)
                    nc.vector.tensor_scalar_add(dst[:], ps[:],
                                                aux_t[:, col:col + 1])
            return emit

        pv_state = {}

        def bg_pv(hl, slot):
            def emit():
                if slot == 0:
                    pv_state[hl] = psV.tile([HD + 1, N], F32, tag="ypv",
                                            name="ypv")
                ypv = pv_state[hl]
                half, mo = slot // 4, (slot % 4) * 2
                for mc in range(mo, mo + 2):
                    nc.tensor.matmul(ypv[:, half * 512:(half + 1) * 512],
                                     vv[mc][:, hl * (HD + 1):
                                            (hl + 1) * (HD + 1)],
                                     e2T[hl % 3][:, mc,
                                                 half * 512:(half + 1) * 512],
                                     start=(mc == 0), stop=(mc == 7))
            return emit

        def bg_zchain(hl):
            def emit():
                ypv = pv_state.pop(hl)
                zr = smS.tile([1, N], F32, tag="zr", name="zr")
                nc.vector.tensor_copy(zr[:], ypv[HD:HD + 1, :])
                rz = smS.tile([1, N], F32, tag="rz", name="rz")
                nc.vector.reciprocal_approx_fast(rz[:], zr[:])
                rzb = smS.tile([64, N], F32, tag="rzb", name="rzb")
                nc.gpsimd.partition_broadcast(rzb[:], rz[:], channels=64)
                c, ro = hl // 2, (hl % 2) * 64
                nc.vector.scalar_tensor_tensor(ymT[c][ro:ro + 64, :],
                                               ypv[0:HD, :],
                                               aux_t[0:64, 8 + hl:9 + hl],
                                               rzb[:],
                                               OP.mult, OP.mult)
            return emit

        # D-phase pools
        smE1 = top.enter_context(tc.tile_pool(name="smE1", bufs=3))
        smT = top.enter_context(tc.tile_pool(name="smT", bufs=3))
        smE2 = top.enter_context(tc.tile_pool(name="smE2", bufs=4))
        smS = top.enter_context(tc.tile_pool(name="smS", bufs=2))

        scols, rscols, rs2cols, pend = {}, {}, {}, {}
        TAYLOR = {1: {1, 3, 5}, 2: {1, 3, 5, 7}, 3: {1, 3}}

        def emit_scores_half(hl, i, half):
            j, ho = hl // 2, (hl % 2) * 64
            if half == 0:
                pend.setdefault(hl, {})[i] = psS.tile([128, N], F32,
                                                      tag="ps_s",
                                                      name="ps_s")
            ps = pend[hl][i]
            nc.tensor.matmul(ps[:, half * 512:(half + 1) * 512],
                             qT[j][ho:ho + 64, i * 128:(i + 1) * 128],
                             kT[j][ho:ho + 64, half * 512:(half + 1) * 512],
                             start=True, stop=True)

        def emit_expchain(hl, i):
            taylor = TAYLOR.get(hl, ())
            if i == 0:
                scols[hl] = smS.tile([128, 8], F32, tag="scol", name="scol")
                rscols[hl] = smS.tile([128, 8], F32, tag="rscol",
                                      name="rscol")
                rs2cols[hl] = smS.tile([128, 8], F32, tag="rs2col",
                                       name="rs2col")
            ps = pend[hl].pop(i)
            e1 = smE1.tile([128, N], BF16, tag="e1", name="e1")
            nc.scalar.activation(e1[:], ps[:], AT.Exp, scale=SCALE,
                                 accum_out=scols[hl][:, i:i + 1])
            nc.vector.reciprocal(rscols[hl][:, i:i + 1],
                                 scols[hl][:, i:i + 1])
            if i in taylor:
                nc.vector.tensor_scalar_mul(rs2cols[hl][:, i:i + 1],
                                            rscols[hl][:, i:i + 1], 0.5)
            t = smT.tile([128, N], BF16, tag="t", name="t")
            nc.vector.tensor_mul(t[:], e1[:], Ct[i][:])
            pend[hl][(i, "t")] = t

        def emit_tail(hl, i):
            taylor = TAYLOR.get(hl, ())
            t = pend[hl].pop((i, "t"))
            e2 = smE2.tile([128, N], BF16, tag="e2", name="e2")
            if i in taylor:
                a = smT.tile([128, N], BF16, tag="ta", name="ta")
                nc.vector.tensor_scalar(a[:], t[:],
                                        rs2cols[hl][:, i:i + 1], 1.0,
                                        OP.mult, OP.add)
                u = smT.tile([128, N], BF16, tag="tu", name="tu")
                nc.vector.tensor_mul(u[:], t[:], a[:])
                nc.vector.tensor_scalar(e2[:], u[:],
                                        rscols[hl][:, i:i + 1], 1.0,
                                        OP.mult, OP.add)
            else:
                nc.scalar.activation(e2[:], t[:], AT.Exp,
                                     scale=rscols[hl][:, i:i + 1])
            nc.sync.dma_start_transpose(
                e2T[hl % 3][:, :, i * 128:(i + 1) * 128], e2[:])

        # slotted two-head pipeline: head h covers slots 4h .. 4h+7
        bgmap = {
            0: [bg_qk1("q", 0)], 1: [bg_qk1("q", 1), bg_v(0)],
            2: [bg_qk1("k", 0), bg_v(1)], 3: [bg_qk1("k", 1), bg_v(2)],
            4: [bg_v(3)], 5: [bg_v(4)], 6: [bg_v(5)], 7: [bg_v(6)],
            8: [bg_v(7)],
            9: [bg_pv(0, 0), bg_pv(0, 1)], 10: [bg_pv(0, 2), bg_pv(0, 3)],
            11: [bg_pv(0, 4), bg_pv(0, 5)], 12: [bg_pv(0, 6), bg_pv(0, 7)],
            13: [bg_zchain(0), bg_pv(1, 0), bg_pv(1, 1)],
            14: [bg_pv(1, 2), bg_pv(1, 3)], 15: [bg_pv(1, 4), bg_pv(1, 5)],
            16: [bg_pv(1, 6), bg_pv(1, 7)],
            17: [bg_zchain(1), bg_pv(2, 0), bg_pv(2, 1)],
            18: [bg_pv(2, 2), bg_pv(2, 3)], 19: [bg_pv(2, 4), bg_pv(2, 5)],
        }

        emit_qk(0, "vector")
        for slot in range(21):
            for emit in bgmap.get(slot, []):
                emit()
            active = [(h, slot - 4 * h) for h in range(4)
                      if 0 <= slot - 4 * h < 8]
            for half in range(2):
                for h, i in active:
                    emit_scores_half(h, i, half)
            for h, i in active:
                emit_expchain(h, i)
            for h, i in active:
                if i >= 1:
                    emit_tail(h, i - 1)
            for h in range(4):
                if slot == 4 * h + 8:
                    emit_tail(h, 7)
        for sl in range(6, 8):
            bg_pv(2, sl)()
        bg_zchain(2)()
        def zchain3_half(half):
            ypv = pv_state[3]
            hs = slice(half * 512, (half + 1) * 512)
            zr = smS.tile([1, 512], F32, tag="zr3", name="zr3")
            nc.vector.tensor_copy(zr[:], ypv[HD:HD + 1, hs])
            rz = smS.tile([1, 512], F32, tag="rz3", name="rz3")
            nc.vector.reciprocal_approx_fast(rz[:], zr[:])
            rzb = smS.tile([64, 512], F32, tag="rzb3", name="rzb3")
            nc.gpsimd.partition_broadcast(rzb[:], rz[:], channels=64)
            nc.vector.scalar_tensor_tensor(ymT[1][64:128, hs],
                                           ypv[0:HD, hs],
                                           aux_t[0:64, 11:12], rzb[:],
                                           OP.mult, OP.mult)

        for sl in range(4):
            bg_pv(3, sl)()
        zchain3_half(0)
        for sl in range(4, 8):
            bg_pv(3, sl)()
        zchain3_half(1)
        pv_state.pop(3)
        mid.close()

        # ---------- phase E: output projection (partial over d-slice) ------
        with tc.tile_pool(name="smE", bufs=3) as smE, \
             tc.tile_pool(name="psE", bufs=3, space="PSUM") as psE:
            for nb in range(8):
                ps = psE.tile([128, DIN], F32, tag="ps_e")
                nc.tensor.matmul(ps[:], ymT[0][:, nb * 128:(nb + 1) * 128],
                                 wp[0][:], start=True, stop=False)
                nc.tensor.matmul(ps[:], ymT[1][:, nb * 128:(nb + 1) * 128],
                                 wp[1][:], start=False, stop=True)
                yo = smE.tile([128, DIN], F32, tag="yo")
                nc.vector.tensor_add(yo[:], ps[:], bp2b[:])
                eng = nc.sync if nb % 2 == 0 else nc.gpsimd
                eng.dma_start(y_d.ap()[nb * 128:(nb + 1) * 128, :], yo[:])

    nc.compile()
    return nc


def make_in_maps(x, compatibility, code, w_c, W_qkv, b_qkv, W_proj, b_proj,
                 ln_qkv_g, ln_qkv_b, ln_proj_g, ln_proj_b):
    import ml_dtypes
    BF = ml_dtypes.bfloat16
    x = np.asarray(x, np.float32)
    c0 = np.asarray(compatibility, np.float32)[0]               # [4,1024]
    CT = np.ascontiguousarray(c0.T @ c0).astype(BF)             # [1024,1024]
    WqT = np.asarray(W_qkv, np.float32).T                        # [512,1536]
    WpT = np.asarray(W_proj, np.float32).T                       # [512,512]
    code = np.asarray(code, np.float32)
    w_c = np.asarray(w_c, np.float32)
    bqkv = np.asarray(b_qkv, np.float32)
    bp2 = (np.asarray(b_proj, np.float32) / 2).reshape(1, DIN).astype(BF)
    lnqg = np.asarray(ln_qkv_g, np.float32)
    lnqb = np.asarray(ln_qkv_b, np.float32)
    lnpg = np.asarray(ln_proj_g, np.float32)
    lnpb = np.asarray(ln_proj_b, np.float32)

    # host-side modulation vectors: cm = layernorm(w_c @ code[:, f]) * g + b
    cm0 = w_c @ code                                             # [512, NF]
    mu = cm0.mean(0, keepdims=True)
    rstd = 1.0 / np.sqrt(cm0.var(0, keepdims=True) + 1e-5)
    cmn = (cm0 - mu) * rstd
    cmq = cmn * lnqg[:, None] + lnqb[:, None]                    # [512, NF]
    cmp = cmn * lnpg[:, None] + lnpb[:, None]

    in_maps = []
    for core in range(N_CORES):
        f, hh = core // 2, core % 2
        dsl = slice(hh * DSL, (hh + 1) * DSL)
        cols = np.r_[hh * DSL:(hh + 1) * DSL,
                     DIN + hh * DSL:DIN + (hh + 1) * DSL,
                     2 * DIN + hh * DSL:2 * DIN + (hh + 1) * DSL]
        bq = bqkv[hh * DSL:(hh + 1) * DSL]
        bk = bqkv[DIN + hh * DSL:DIN + (hh + 1) * DSL]
        aux = np.zeros((128, 12), np.float32)
        aux[:, 0] = bq[0:128]
        aux[:, 1] = bq[128:256]
        aux[:, 2] = bk[0:128]
        aux[:, 3] = bk[128:256]
        aux[:, 4:8] = cmq[:, f].reshape(4, 128).T
        aux[0:64, 8:12] = cmp[dsl, f].reshape(4, 64).T
        xmT = np.ascontiguousarray((x[0] * cmq[None, :, f]).T).astype(BF)
        in_maps.append(dict(
            xmT=xmT,
            ct=CT,
            wqkvT=np.ascontiguousarray(WqT[:, cols]).astype(BF),
            wprojT=np.ascontiguousarray(WpT[dsl, :]).astype(BF),
            aux=aux,
            bv=bqkv[2 * DIN + hh * DSL:2 * DIN + (hh + 1) * DSL]
                .reshape(1, DSL).astype(BF),
            bp2=bp2,
        ))
    return in_maps


def kernel(**inputs) -> np.ndarray:
    from concourse.bass_utils import run_bass_kernel_spmd
    if "nc" not in _CACHE:
        _CACHE["nc"] = build_nc()
    nc = _CACHE["nc"]
    in_maps = make_in_maps(**inputs)
    res = run_bass_kernel_spmd(nc, in_maps, core_ids=list(range(N_CORES)))
    out = np.empty((1, NF, N, DIN), np.float32)
    for f in range(NF):
        out[0, f] = (np.asarray(res.results[2 * f]["y"], np.float32)
                     + np.asarray(res.results[2 * f + 1]["y"], np.float32))
    return out


# revision 41
# speedup vs baseline: 1.0573x; 1.0573x over previous
"""Trainium2 Bass kernel for nn_ModAttn (modulated multi-function attention).

Shapes: x [1,1024,512], compatibility [1,4,1024]; out [1,4,1024,512].
Sharding: 8 cores = (function f in 0..3) x (head-half hh in 0..1). Each core
computes 4 heads (global heads hh*4..hh*4+3) over all 1024 queries/keys and
produces a PARTIAL output projection over its 256 d-dims; the host sums the
two partials per function (each core adds b_proj/2).

Per-core dataflow, attention in NORMAL orientation (query n on partitions):
  cm_q/cm_p   = layernorm(w_c @ code_f) modulation rows (phase A)
  C           = comp^T comp tiles [n-chunk 128, m 1024] bf16 (phase C)
  xm          = xT * cm_q (per-partition, 4x DVE)            (phase B)
  qT,kT       = [128=2 heads x 64d, 1024 tok] bf16; v dense [m-chunk, 256]
  per head (pipelined):
    S         = qT-slice^T @ kT-slice -> PSUM [128 n, 1024 m]
    e1        = ACT Exp(SCALE*S), accum_out -> s (softmax-1 denominators)
    t         = e1 * C[n-chunk]         (DVE 2x)
    e2        = ACT Exp(t * (1/s) per-partition scale), accum_out -> z2
    e2n       = e2 * (1/z2)             (DVE tensor_scalar 4x)
    e2T       = DMA-engine xbar transpose -> [m-local, m-chunk, n]
    ypv       = v^T @ e2T (PSUM accum over m-chunks)
    ymT       = ypv * cm_p-col          (DVE per-partition)
  y_partial   = ymT^T @ W_proj-slice + b_proj/2 -> HBM fp32   (phase E)
"""

import numpy as np
from contextlib import ExitStack

N_CORES = 8
N, DIN, NF, H, HD = 1024, 512, 4, 8, 64
HH = 4          # heads per core
DSL = 256       # qkv d-dims per core
SCALE = HD ** -0.5

_CACHE = {}


def build_nc():
    import concourse.bacc as bacc
    import concourse.tile as tile
    from concourse import mybir

    F32 = mybir.dt.float32
    BF16 = mybir.dt.bfloat16
    AT = mybir.ActivationFunctionType
    OP = mybir.AluOpType

    nc = bacc.Bacc("TRN2", target_bir_lowering=False, debug=False,
                   num_devices=N_CORES)

    xmT_d = nc.dram_tensor("xmT", [DIN, N], BF16, kind="ExternalInput")
    ct_d = nc.dram_tensor("ct", [N, N], BF16, kind="ExternalInput")
    wqkvT_d = nc.dram_tensor("wqkvT", [DIN, 3 * DSL], BF16,
                             kind="ExternalInput")
    wprojT_d = nc.dram_tensor("wprojT", [DSL, DIN], BF16,
                              kind="ExternalInput")
    aux_d = nc.dram_tensor("aux", [128, 12], F32, kind="ExternalInput")
    bv_d = nc.dram_tensor("bv", [1, DSL], BF16, kind="ExternalInput")
    bp2_d = nc.dram_tensor("bp2", [1, DIN], BF16, kind="ExternalInput")
    y_d = nc.dram_tensor("y", [N, DIN], F32, kind="ExternalOutput")

    with tile.TileContext(nc) as tc, ExitStack() as top:
        const = top.enter_context(tc.tile_pool(name="const", bufs=1))
        ones_r = const.tile([1, 128], F32, tag="ones_r")
        nc.vector.memset(ones_r[:], 1.0)
        ones_rb = const.tile([1, 128], BF16, tag="ones_rb")
        nc.vector.memset(ones_rb[:], 1.0)
        warm = const.tile([1, 1], F32, tag="warm")
        nc.scalar.activation(warm[:], ones_r[0:1, 0:1], AT.Exp)
        F32R = mybir.dt.float32r
        ones_rr = const.tile([1, 64], F32R, tag="ones_rr")
        nc.vector.memset(ones_rr[:].bitcast(F32), 1.0)

        big = top.enter_context(tc.tile_pool(name="big", bufs=1))
        xm = [big.tile([128, N], BF16, tag=f"xm{c}", name=f"xm{c}")
              for c in range(4)]
        wq = [big.tile([128, 3 * DSL], BF16, tag=f"wq{c}", name=f"wq{c}")
              for c in range(4)]
        wp = [big.tile([128, DIN], BF16, tag=f"wp{c}", name=f"wp{c}")
              for c in range(2)]
        for c in range(4):
            nc.gpsimd.dma_start(xm[c][:], xmT_d.ap()[c * 128:(c + 1) * 128, :])
        for c in range(4):
            nc.gpsimd.dma_start(wq[c][:], wqkvT_d.ap()[c * 128:(c + 1) * 128, :])
        aux_t = big.tile([128, 12], F32, tag="aux_t")
        nc.sync.dma_start(aux_t[:], aux_d.ap())

        bv_t = big.tile([1, DSL], BF16, tag="bv_t")
        nc.sync.dma_start(bv_t[:], bv_d.ap())
        bp2_t = big.tile([1, DIN], BF16, tag="bp2_t")
        nc.sync.dma_start(bp2_t[:], bp2_d.ap())
        bp2b = big.tile([128, DIN], BF16, tag="bp2b")
        nc.gpsimd.partition_broadcast(bp2b[:], bp2_t[:], channels=128)
        for c in range(2):
            nc.gpsimd.dma_start(wp[c][:], wprojT_d.ap()[c * 128:(c + 1) * 128, :])

        # persistent attention operands
        qkv = top.enter_context(tc.tile_pool(name="qkv", bufs=1))
        qT = [qkv.tile([128, N], BF16, tag=f"qT{j}", name=f"qT{j}")
              for j in range(2)]
        kT = [qkv.tile([128, N], BF16, tag=f"kT{j}", name=f"kT{j}")
              for j in range(2)]
        vv = [qkv.tile([128, HH * (HD + 1)], BF16, tag=f"vv{m}",
                       name=f"vv{m}") for m in range(8)]
        Ct = [qkv.tile([128, N], BF16, tag=f"Ct{n}", name=f"Ct{n}")
              for n in range(8)]
        ymT = [qkv.tile([128, N], BF16, tag=f"ymT{c}", name=f"ymT{c}")
               for c in range(2)]
        e2T = [qkv.tile([128, 8, N], BF16, tag=f"e2T{b}", name=f"e2T{b}")
               for b in range(3)]
        for ncb in range(8):
            nc.gpsimd.dma_start(Ct[ncb][:],
                                ct_d.ap()[ncb * 128:(ncb + 1) * 128, :])

        # ---------- phase B + D interleaved ----------
        mid = ExitStack()
        psS = mid.enter_context(tc.tile_pool(name="psS", bufs=3, space="PSUM"))
        psV = mid.enter_context(tc.tile_pool(name="psV", bufs=1, space="PSUM"))

        qk0_ps = {}

        def emit_qk_half(j, which, half):
            dst, off, col = ((qT[j], j * 128, j) if which == "q"
                             else (kT[j], DSL + j * 128, 2 + j))
            key = (j, which)
            if half == 0:
                qk0_ps[key] = psS.tile([128, N], F32, tag="ps_s",
                                       name="ps_s")
            ps = qk0_ps[key]
            hs = slice(half * 512, (half + 1) * 512)
            for c in range(4):
                nc.tensor.matmul(ps[:, hs], wq[c][:, off:off + 128],
                                 xm[c][:, hs], start=(c == 0), stop=(c == 3))
            nc.vector.tensor_scalar_add(dst[:, hs], ps[:, hs],
                                        aux_t[:, col:col + 1])

        def emit_qk(j, engine):
            for half in range(2):
                emit_qk_half(j, "k", half)
                emit_qk_half(j, "q", half)

        # --- background work generators (emitted between D1 chunks) ---

        def bg_v(mc):
            def emit():
                ps = psS.tile([128, N], F32, tag="ps_s", name="ps_s")
                pv = ps[:, 0:DSL]
                for c in range(4):
                    nc.tensor.matmul(pv, xm[c][:, mc * 128:(mc + 1) * 128],
                                     wq[c][:, 2 * DSL:3 * DSL],
                                     start=(c == 0), stop=False)
                nc.tensor.matmul(pv, ones_rb[:], bv_t[:], start=False,
                                 stop=True)
                v3 = vv[mc][:].rearrange("p (h e) -> p h e", e=HD + 1)
                nc.vector.tensor_copy(
                    v3[:, :, 0:HD], pv.rearrange("p (h e) -> p h e", e=HD))
                nc.vector.memset(v3[:, :, HD:HD + 1], 1.0)
            return emit

        qk1_state = {}

        def bg_qk1(which, half):
            off = 128 if which == "q" else DSL + 128
            def emit():
                if half == 0:
                    qk1_state[which] = psS.tile([128, N], F32, tag="ps_s",
                                                name="ps_s")
                ps = qk1_state[which]
                for c in range(4):
                    nc.tensor.matmul(ps[:, half * 512:(half + 1) * 512],
                                     wq[c][:, off:off + 128],
                                     xm[c][:, half * 512:(half + 1) * 512],
                                     start=(c == 0), stop=(c == 3))
                if half == 1:
                    dst, col = (qT[1], 1) if which == "q" else (kT[1], 3)
                    nc.vector.tensor_scalar_add(dst[:], ps[:],
                                                aux_t[:, col:col + 1])
            return emit

        pv_state = {}

        def bg_pv(hl, slot):
            def emit():
                if slot == 0:
                    pv_state[hl] = psV.tile([HD + 1, N], F32, tag="ypv",
                                            name="ypv")
                ypv = pv_state[hl]
                half, mo = slot // 4, (slot % 4) * 2
                for mc in range(mo, mo + 2):
                    nc.tensor.matmul(ypv[:, half * 512:(half + 1) * 512],
                                     vv[mc][:, hl * (HD + 1):
                                            (hl + 1) * (HD + 1)],
                                     e2T[hl % 3][:, mc,
                                                 half * 512:(half + 1) * 512],
                                     start=(mc == 0), stop=(mc == 7))
            return emit

        def bg_zchain(hl):
            def emit():
                ypv = pv_state.pop(hl)
                zr = smS.tile([1, N], F32, tag="zr", name="zr")
                nc.vector.tensor_copy(zr[:], ypv[HD:HD + 1, :])
                rz = smS.tile([1, N], F32, tag="rz", name="rz")
                nc.vector.reciprocal_approx_fast(rz[:], zr[:])
                rzb = smS.tile([64, N], F32, tag="rzb", name="rzb")
                nc.gpsimd.partition_broadcast(rzb[:], rz[:], channels=64)
                c, ro = hl // 2, (hl % 2) * 64
                nc.vector.scalar_tensor_tensor(ymT[c][ro:ro + 64, :],
                                               ypv[0:HD, :],
                                               aux_t[0:64, 8 + hl:9 + hl],
                                               rzb[:],
                                               OP.mult, OP.mult)
            return emit

        # D-phase pools
        smE1 = top.enter_context(tc.tile_pool(name="smE1", bufs=3))
        smT = top.enter_context(tc.tile_pool(name="smT", bufs=3))
        smE2 = top.enter_context(tc.tile_pool(name="smE2", bufs=4))
        smS = top.enter_context(tc.tile_pool(name="smS", bufs=2))

        scols, rscols, rs2cols, pend = {}, {}, {}, {}
        TAYLOR = {1: {1, 3, 5}, 2: {1, 3, 5, 7}, 3: {1, 3}}

        def emit_scores_half(hl, i, half):
            j, ho = hl // 2, (hl % 2) * 64
            if half == 0:
                pend.setdefault(hl, {})[i] = psS.tile([128, N], F32,
                                                      tag="ps_s",
                                                      name="ps_s")
            ps = pend[hl][i]
            nc.tensor.matmul(ps[:, half * 512:(half + 1) * 512],
                             qT[j][ho:ho + 64, i * 128:(i + 1) * 128],
                             kT[j][ho:ho + 64, half * 512:(half + 1) * 512],
                             start=True, stop=True)

        def emit_expchain(hl, i):
            taylor = TAYLOR.get(hl, ())
            if i == 0:
                scols[hl] = smS.tile([128, 8], F32, tag="scol", name="scol")
                rscols[hl] = smS.tile([128, 8], F32, tag="rscol",
                                      name="rscol")
                rs2cols[hl] = smS.tile([128, 8], F32, tag="rs2col",
                                       name="rs2col")
            ps = pend[hl].pop(i)
            e1 = smE1.tile([128, N], BF16, tag="e1", name="e1")
            nc.scalar.activation(e1[:], ps[:], AT.Exp, scale=SCALE,
                                 accum_out=scols[hl][:, i:i + 1])
            nc.vector.reciprocal(rscols[hl][:, i:i + 1],
                                 scols[hl][:, i:i + 1])
            if i in taylor:
                nc.vector.tensor_scalar_mul(rs2cols[hl][:, i:i + 1],
                                            rscols[hl][:, i:i + 1], 0.5)
            t = smT.tile([128, N], BF16, tag="t", name="t")
            nc.vector.tensor_mul(t[:], e1[:], Ct[i][:])
            pend[hl][(i, "t")] = t

        def emit_tail(hl, i):
            taylor = TAYLOR.get(hl, ())
            t = pend[hl].pop((i, "t"))
            e2 = smE2.tile([128, N], BF16, tag="e2", name="e2")
            if i in taylor:
                a = smT.tile([128, N], BF16, tag="ta", name="ta")
                nc.vector.tensor_scalar(a[:], t[:],
                                        rs2cols[hl][:, i:i + 1], 1.0,
                                        OP.mult, OP.add)
                u = smT.tile([128, N], BF16, tag="tu", name="tu")
                nc.vector.tensor_mul(u[:], t[:], a[:])
                nc.vector.tensor_scalar(e2[:], u[:],
                                        rscols[hl][:, i:i + 1], 1.0,
                                        OP.mult, OP.add)
            else:
                nc.scalar.activation(e2[:], t[:], AT.Exp,
                                     scale=rscols[hl][:, i:i + 1])
            nc.sync.dma_start_transpose(
                e2T[hl % 3][:, :, i * 128:(i + 1) * 128], e2[:])

        # slotted two-head pipeline: head h covers slots 4h .. 4h+7
        bgmap = {
            0: [bg_qk1("q", 0)], 1: [bg_qk1("q", 1), bg_v(0)],
            2: [bg_qk1("k", 0), bg_v(1)], 3: [bg_qk1("k", 1), bg_v(2)],
            4: [bg_v(3)], 5: [bg_v(4)], 6: [bg_v(5)], 7: [bg_v(6)],
            8: [bg_v(7)],
            9: [bg_pv(0, 0), bg_pv(0, 1)], 10: [bg_pv(0, 2), bg_pv(0, 3)],
            11: [bg_pv(0, 4), bg_pv(0, 5)], 12: [bg_pv(0, 6), bg_pv(0, 7)],
            13: [bg_zchain(0), bg_pv(1, 0), bg_pv(1, 1)],
            14: [bg_pv(1, 2), bg_pv(1, 3)], 15: [bg_pv(1, 4), bg_pv(1, 5)],
            16: [bg_pv(1, 6), bg_pv(1, 7)],
            17: [bg_zchain(1), bg_pv(2, 0), bg_pv(2, 1)],
            18: [bg_pv(2, 2), bg_pv(2, 3)], 19: [bg_pv(2, 4), bg_pv(2, 5)],
        }

        emit_qk(0, "vector")
        for slot in range(21):
            for emit in bgmap.get(slot, []):
                emit()
            active = [(h, slot - 4 * h) for h in range(4)
                      if 0 <= slot - 4 * h < 8]
            for half in range(2):
                for h, i in active:
                    emit_scores_half(h, i, half)
            for h, i in active:
                emit_expchain(h, i)
            for h, i in active:
                if i >= 1:
                    emit_tail(h, i - 1)
            for h in range(4):
                if slot == 4 * h + 8:
                    emit_tail(h, 7)
        for sl in range(6, 8):
            bg_pv(2, sl)()
        bg_zchain(2)()
        def zchain3_half(half):
            ypv = pv_state[3]
            hs = slice(half * 512, (half + 1) * 512)
            zr = smS.tile([1, 512], F32, tag="zr3", name="zr3")
            nc.vector.tensor_copy(zr[:], ypv[HD:HD + 1, hs])
            rz = smS.tile([1, 512], F32, tag="rz3", name="rz3")
            nc.vector.reciprocal_approx_fast(rz[:], zr[:])
            rzb = smS.tile([64, 512], F32, tag="rzb3", name="rzb3")
            nc.gpsimd.partition_broadcast(rzb[:], rz[:], channels=64)
            nc.vector.scalar_tensor_tensor(ymT[1][64:128, hs],
                                           ypv[0:HD, hs],
                                           aux_t[0:64, 11:12], rzb[:],
                                           OP.mult, OP.mult)

        for sl in range(4):
            bg_pv(3, sl)()
        zchain3_half(0)
        for sl in range(4, 8):
            bg_pv(3, sl)()
        zchain3_half(1)
        pv_state.pop(3)
        mid.close()

        # ---------- phase E: output projection (partial over d-slice) ------
        with tc.tile_pool(name="smE", bufs=3) as smE, \
             tc.tile_pool(name="psE", bufs=3, space="PSUM") as psE:
            for nb in range(8):
                ps = psE.tile([128, DIN], F32, tag="ps_e")
                nc.tensor.matmul(ps[:], ymT[0][:, nb * 128:(nb + 1) * 128],
                                 wp[0][:], start=True, stop=False)
                nc.tensor.matmul(ps[:], ymT[1][:, nb * 128:(nb + 1) * 128],
                                 wp[1][:], start=False, stop=True)
                yo = smE.tile([128, DIN], F32, tag="yo")
                nc.vector.tensor_add(yo[:], ps[:], bp2b[:])
                eng = nc.sync if nb % 2 == 0 else nc.gpsimd
                eng.dma_start(y_d.ap()[nb * 128:(nb + 1) * 128, :], yo[:])

    nc.compile()
    return nc


def make_in_maps(x, compatibility, code, w_c, W_qkv, b_qkv, W_proj, b_proj,
                 ln_qkv_g, ln_qkv_b, ln_proj_g, ln_proj_b):
    import ml_dtypes
    BF = ml_dtypes.bfloat16
    x = np.asarray(x, np.float32)
    c0 = np.asarray(compatibility, np.float32)[0]               # [4,1024]
    CT = np.ascontiguousarray(c0.T @ c0).astype(BF)             # [1024,1024]
    WqT = np.asarray(W_qkv, np.float32).T                        # [512,1536]
    WpT = np.asarray(W_proj, np.float32).T                       # [512,512]
    code = np.asarray(code, np.float32)
    w_c = np.asarray(w_c, np.float32)
    bqkv = np.asarray(b_qkv, np.float32)
    bp2 = (np.asarray(b_proj, np.float32) / 2).reshape(1, DIN).astype(BF)
    lnqg = np.asarray(ln_qkv_g, np.float32)
    lnqb = np.asarray(ln_qkv_b, np.float32)
    lnpg = np.asarray(ln_proj_g, np.float32)
    lnpb = np.asarray(ln_proj_b, np.float32)

    # host-side modulation vectors: cm = layernorm(w_c @ code[:, f]) * g + b
    cm0 = w_c @ code                                             # [512, NF]
    mu = cm0.mean(0, keepdims=True)
    rstd = 1.0 / np.sqrt(cm0.var(0, keepdims=True) + 1e-5)
    cmn = (cm0 - mu) * rstd
    cmq = cmn * lnqg[:, None] + lnqb[:, None]                    # [512, NF]
    cmp = cmn * lnpg[:, None] + lnpb[:, None]

    in_maps = []
    for core in range(N_CORES):
        f, hh = core // 2, core % 2
        dsl = slice(hh * DSL, (hh + 1) * DSL)
        cols = np.r_[hh * DSL:(hh + 1) * DSL,
                     DIN + hh * DSL:DIN + (hh + 1) * DSL,
                     2 * DIN + hh * DSL:2 * DIN + (hh + 1) * DSL]
        bq = bqkv[hh * DSL:(hh + 1) * DSL]
        bk = bqkv[DIN + hh * DSL:DIN + (hh + 1) * DSL]
        aux = np.zeros((128, 12), np.float32)
        aux[:, 0] = bq[0:128]
        aux[:, 1] = bq[128:256]
        aux[:, 2] = bk[0:128]
        aux[:, 3] = bk[128:256]
        aux[:, 4:8] = cmq[:, f].reshape(4, 128).T
        aux[0:64, 8:12] = cmp[dsl, f].reshape(4, 64).T
        xmT = np.ascontiguousarray((x[0] * cmq[None, :, f]).T).astype(BF)
        in_maps.append(dict(
            xmT=xmT,
            ct=CT,
            wqkvT=np.ascontiguousarray(WqT[:, cols]).astype(BF),
            wprojT=np.ascontiguousarray(WpT[dsl, :]).astype(BF),
            aux=aux,
            bv=bqkv[2 * DIN + hh * DSL:2 * DIN + (hh + 1) * DSL]
                .reshape(1, DSL).astype(BF),
            bp2=bp2,
        ))
    return in_maps


def kernel(**inputs) -> np.ndarray:
    from concourse.bass_utils import run_bass_kernel_spmd
    if "nc" not in _CACHE:
        _CACHE["nc"] = build_nc()
    nc = _CACHE["nc"]
    in_maps = make_in_maps(**inputs)
    res = run_bass_kernel_spmd(nc, in_maps, core_ids=list(range(N_CORES)))
    out = np.empty((1, NF, N, DIN), np.float32)
    for f in range(NF):
        out[0, f] = (np.asarray(res.results[2 * f]["y"], np.float32)
                     + np.asarray(res.results[2 * f + 1]["y"], np.float32))
    return out
